# revision 1
# baseline (speedup 1.0000x reference)
"""Trainium2 Bass kernel: boson-sampler probabilities via Glynn's permanent formula.

Math (per 18x18 complex matrix A):
  perm(A) = 2^(1-n) * sum_{d in {+-1}^n, d_0=+1} (prod_k d_k) * prod_i (sum_j d_j A[i,j])
The 2^17 sign vectors form a [128 x 1024] grid (7 "p" bits drive columns 11..17,
10 "f" bits drive columns 1..10; column 0 fixed +1). Row-sums factor as
rs_i = RP_i(p) + RF_i(f); rows are grouped [6,6,6] and each group's product
expands as T_g[p,f] = sum_{c<64} G_g[c,p] * H_g[c,f] -- K=64 fp32 matmuls on the
tensor engine. The 64-row G/H tables (all sub-products of 6 rows) are built
hierarchically (pairs -> quads -> tables): packed operand sets are assembled
from an SBUF "master" row tile by 0/1 selection matmuls (exact in fp32), and
each level is one set of 6 vector-engine tensor ops (complex multiply). Glynn
parity signs are folded into group 2's tables via sign-scaled mask constants.
The final product T0*T1*T2 collapses through a fused scalar_tensor_tensor
reduce and a K=128 ones-matmul. One NeuronCore per batch element.
"""

import sys

sys.path.insert(0, "/opt/trn_rl_repo")

import numpy as np

import concourse.bacc as bacc
import concourse.bass as bass
import concourse.tile as tile
from concourse import mybir
from concourse.bass_utils import run_bass_kernel_spmd

FP32 = mybir.dt.float32
OP = mybir.AluOpType

N = 18
PBITS, FBITS = 7, 10
P, F = 1 << PBITS, 1 << FBITS          # 128, 1024
EMU = 0.85 * (1 - 0.02) * (1 - 0.02) * (1 - 0.01)
DARK = 1e-6 * N
SCALE2 = float(2.0 ** (2 * (1 - N)))

# master row map (same for both sides; imag comp has zeros at ONES/SIGN)
M_RF = 0          # rows 0..17: row-sums RF_i / RP_i
M_SE = 18         # sign-scaled even row of pair 8 (RFe' / RPe')
M_SO = 19         # sign-scaled odd row (RFo' / RPo')
M_ONE = 20        # ones (real) / zeros (imag)
M_SGN = 21        # sign row sF / sP (real) / zeros (imag)
M_PP = 32         # rows 32..40: pair products PP_q
M_PPS = 41        # row 41: PP' = sign-scaled pair-8 product
M_QD = 64         # rows 64..90: quad mult rows (9 per group, za-fast)
M_ROWS = 91
M_PAD = 96

_CACHE = {}


def _pm_mask(nvals, bits):
    v = np.arange(nvals, dtype=np.uint32)
    m = (v[:, None] >> np.arange(bits, dtype=np.uint32)[None, :]) & 1
    return (1.0 - 2.0 * m).astype(np.float32).T.copy()   # [bits, nvals]


def _parity(nvals, bits):
    v = np.arange(nvals, dtype=np.uint32)
    pc = np.zeros(nvals, dtype=np.uint32)
    for k in range(bits):
        pc += (v >> k) & 1
    return np.where(pc % 2 == 0, 1.0, -1.0).astype(np.float32)


class SideSpec:
    """H: f-side (width 1024, pair-table identity at z=3);
       G: p-side (width 128, identity at z=0)."""

    def __init__(self, name, width, mult_z):
        self.name = name
        self.w = width
        self.mult_z = list(mult_z)
        self.idz = ({0, 1, 2, 3} - set(mult_z)).pop()

    # pair-table entry -> master row (pair q, entry z); identity z -> ones row
    def tab_row(self, q, z):
        if z == self.idz:
            return M_ONE
        if self.name == "H":
            return {0: M_PP + q, 1: 2 * q + 1, 2: 2 * q}[z]
        return {1: 2 * q, 2: 2 * q + 1, 3: M_PP + q}[z]

    # group-2 scaled pair-table entry (pair 8) -> master row
    def tab2_row(self, z):
        if z == self.idz:
            return M_SGN
        if self.name == "H":
            return {0: M_PPS, 1: M_SO, 2: M_SE}[z]
        return {1: M_SE, 2: M_SO, 3: M_PPS}[z]

    # quad-table row (group g, index c2 = za + 4*zb) -> master row
    def quad_row(self, g, c2):
        za, zb = c2 % 4, c2 // 4
        qa, qb = 3 * g, 3 * g + 1
        if za != self.idz and zb != self.idz:
            ia = self.mult_z.index(za)
            ib = self.mult_z.index(zb)
            return M_QD + 9 * g + 3 * ib + ia
        if za == self.idz and zb == self.idz:
            return M_ONE
        if za == self.idz:
            return self.tab_row(qb, zb)
        return self.tab_row(qa, za)


HSPEC = SideSpec("H", F, (0, 1, 2))
GSPEC = SideSpec("G", P, (1, 2, 3))


def _sel(rows, m_pad=None):
    """Selection matrix [M_PAD, len(rows)] with one 1 per used column."""
    M = len(rows) if m_pad is None else m_pad
    s = np.zeros((M_PAD, M), np.float32)
    for m, k in enumerate(rows):
        if k is not None:
            s[k, m] = 1.0
    return s


# const pack column layout: computed once at import
def _build_pack():
    cols = {}
    blocks = []
    off = 0

    def add(name, arr):
        nonlocal off
        a = np.zeros((M_PAD, arr.shape[1]), np.float32)
        a[0:arr.shape[0], :] = arr
        cols[name] = (off, arr.shape[1])
        blocks.append(a)
        off += arr.shape[1]

    # MFX: [ones;pm(10) | (ones;pm)*sF] (row 0 = ones source, row 11 = sF source)
    mF = np.concatenate([np.ones((1, F), np.float32), _pm_mask(F, FBITS)], axis=0)
    sF = _parity(F, FBITS)
    add("MFX", np.concatenate([mF, mF * sF[None, :]], axis=0))           # [22, F]
    # MPX: [pm(7) | pm*sP | ones | sP]
    mP = _pm_mask(P, PBITS)
    sP = _parity(P, PBITS)
    add("MPX", np.concatenate(
        [mP, mP * sP[None, :], np.ones((1, P), np.float32), sP[None, :]], axis=0))
    # unit columns for the widened RS matmul lhsT (cols 20,21), per side
    uh = np.zeros((22, 2), np.float32)
    uh[0, 0] = 1.0      # -> MFX row 0 (ones)
    uh[11, 1] = 1.0     # -> MFX row 11 (sF)
    add("U_H", uh)
    ug = np.zeros((16, 2), np.float32)
    ug[14, 0] = 1.0     # -> MPX row 14 (ones)
    ug[15, 1] = 1.0     # -> MPX row 15 (sP)
    add("U_G", ug)

    def digits(c):
        return c % 4, (c // 4) % 4, c // 16          # za, zb, zc

    for spec in (HSPEC, GSPEC):
        nm = spec.name
        # pair stage: in0 rows [evens, RFe'], in1 rows [odds, RF_17]
        in0 = [2 * q for q in range(9)] + [M_SE]
        in1 = [2 * q + 1 for q in range(9)] + [17]
        add(f"SELP_{nm}", _sel(in0 + [None] * 22 + in1, 42))
        # L1: 48 distinct products tmp48[16g + za-idx + 4*zb-idx... indexed by
        # (g, c2) with c2 = za + 4zb: in0/in1 packed [48 | pad | 48] in one mm
        in0 = []
        in1 = []
        for g in range(3):
            for c2 in range(16):
                in0.append(spec.tab_row(3 * g, c2 % 4))
                in1.append(spec.tab_row(3 * g + 1, c2 // 4))
        add(f"SELL1_{nm}", _sel(in0 + [None] * 16 + in1, 112))
        # REP: replicate tmp48 rows into L2 src layout (lhsT for rep matmuls)
        # repA: [64g + c] <- tmp48[16g + (c % 16)] for g=0,1 ; repB: g=2
        repA = np.zeros((48, 128), np.float32)
        for g in range(2):
            for c in range(64):
                repA[16 * g + (c % 16), 64 * g + c] = 1.0
        add(f"REPA_{nm}", repA)
        repB = np.zeros((48, 64), np.float32)
        for c in range(64):
            repB[32 + (c % 16), c] = 1.0
        add(f"REPB_{nm}", repB)
        # L2 c-packs: set1 rows [64g+c] = tab_c[zc]; set2 = scaled tab2'[zc]
        rows = []
        for g in range(2):
            for c in range(64):
                rows.append(spec.tab_row(3 * g + 2, digits(c)[2]))
        add(f"SELL2A_{nm}", _sel(rows))
        rows = [spec.tab2_row(digits(c)[2]) for c in range(64)]
        add(f"SELL2B_{nm}", _sel(rows))
    return np.concatenate(blocks, axis=1), cols


CPACK, CPACK_COLS = _build_pack()
CW = CPACK.shape[1]


def host_consts():
    return {"CPACK": CPACK}


# ---------------------------------------------------------------- kernel body
def build_kernel(loop_iters=None):
    nc = bacc.Bacc("TRN2", target_bir_lowering=False, debug=False)

    tens = {}
    tens["A_real"] = nc.dram_tensor("A_real", [N, N], FP32, kind="ExternalInput").ap()
    tens["A_imag"] = nc.dram_tensor("A_imag", [N, N], FP32, kind="ExternalInput").ap()
    tens["CPACK"] = nc.dram_tensor("CPACK", [M_PAD, CW], FP32,
                                   kind="ExternalInput").ap()
    tens["OUT"] = nc.dram_tensor("OUT", [1, 1], FP32, kind="ExternalOutput").ap()
    tens["sq_dram"] = nc.dram_tensor("sq_dram", [1, N * N], FP32).ap()

    with tile.TileContext(nc) as tc:
        if loop_iters is None:
            _body(nc, tc, tens)
        else:
            with tc.For_i(0, loop_iters, 1):
                _body(nc, tc, tens)
    nc.compile()
    return nc


def _body(nc, tc, tens):
    from contextlib import ExitStack

    ctx = ExitStack()
    pers = ctx.enter_context(tc.tile_pool(name="pers", bufs=1))
    pk = ctx.enter_context(tc.tile_pool(name="pk", bufs=2))
    cm = ctx.enter_context(tc.tile_pool(name="cm", bufs=2))
    psum_pool = ctx.enter_context(tc.tile_pool(name="psum", bufs=4, space="PSUM"))
    dma = nc.sync.dma_start
    dma2 = nc.gpsimd.dma_start          # SWDGE queues for small input loads

    def cmul6(rows, w, i0, i1, outr, outi, eng=None):
        """(i0r,i0i)*(i1r,i1i) -> (outr,outi) APs; 6 elementwise TT ops."""
        e = eng or nc.vector
        i0r, i0i = i0
        i1r, i1i = i1
        tg = "gp_" if eng is not None else "cm_"
        t1 = cm.tile([rows, w], FP32, tag=f"{tg}t1")
        t2 = cm.tile([rows, w], FP32, tag=f"{tg}t2")
        e.tensor_mul(t1[:], i0r, i1r)
        e.tensor_mul(t2[:], i0i, i1i)
        e.tensor_sub(outr, t1[:], t2[:])
        e.tensor_mul(t1[:], i0r, i1i)
        e.tensor_mul(t2[:], i0i, i1r)
        e.tensor_add(outi, t1[:], t2[:])

    def sel_mm(sel_sb, msrc, m, w):
        """Pack = SEL.T @ master -> PSUM [m, w]."""
        ps = psum_pool.tile([m, w], FP32, tag="ps")
        for c0 in range(0, w, 512):
            c1 = min(c0 + 512, w)
            nc.tensor.matmul(ps[:, c0:c1], sel_sb[:], msrc[:, c0:c1],
                             start=True, stop=True)
        return ps

    # ---- stage 0: A loads, widened row-sum matmuls -> master rows 0..21
    # lhsT layout per side (K rows x M=22 cols):
    #   rows 0..kb-1: A^T slice (side's columns) -> outputs 0..17 = row-sums
    #   rows kb..2kb-1 at cols 18,19: A^T cols 16,17 -> sign-scaled rows
    #   unit entries select mask ones/sign rows -> outputs 20 (ones), 21 (sign)
    cpk = pers.tile([M_PAD, CW], FP32, tag="cpack")
    dma(cpk[:], tens["CPACK"][:, :])

    def cslice(name, nrows=None):
        off, width = CPACK_COLS[name]
        nr = M_PAD if nrows is None else nrows
        return cpk[0:nr, off:off + width]

    sbA = {}
    lhsT_rs = {}
    KRS = {"H": 22, "G": 16}
    for nm, src in (("r", tens["A_real"]), ("i", tens["A_imag"])):
        a = pers.tile([N, N], FP32, tag=f"A{nm}")
        dma2(a[:], src[:, :])
        sbA[nm] = a
        for spec, kb in ((HSPEC, 11), (GSPEC, 7)):
            lo = 0 if spec.name == "H" else 11
            K_rs = KRS[spec.name]
            lt = pers.tile([K_rs, 22], FP32, tag=f"lt{spec.name}{nm}")
            nc.gpsimd.memset(lt[:], 0.0)
            dma2(lt[0:kb, 0:N], src.transpose((1, 0))[lo:lo + kb, :])
            dma2(lt[kb:2 * kb, 18:20], src.transpose((1, 0))[lo:lo + kb, 16:18])
            if nm == "r":
                nc.vector.tensor_copy(lt[:, 20:22],
                                      cslice(f"U_{spec.name}", K_rs))
            lhsT_rs[(spec.name, nm)] = lt

    mask_sb = {"H": cslice("MFX", 22), "G": cslice("MPX", 16)}
    sel_sb = {}
    for spec in (HSPEC, GSPEC):
        for s in ("SELP", "SELL1", "SELL2A", "SELL2B"):
            key = f"{s}_{spec.name}"
            sel_sb[key] = cslice(key)
        for s in ("REPA", "REPB"):
            key = f"{s}_{spec.name}"
            sel_sb[key] = cslice(key, 48)

    master = {}
    for spec in (HSPEC, GSPEC):
        for nm in "ri":
            t = pers.tile([M_PAD, spec.w], FP32, tag=f"mst{spec.name}{nm}")
            nc.gpsimd.memset(t[:], 0.0)
            master[(spec.name, nm)] = t

    for spec in (HSPEC, GSPEC):
        for nm in "ri":
            lt = lhsT_rs[(spec.name, nm)]
            ps = psum_pool.tile([22, spec.w], FP32, tag="ps")
            for c0 in range(0, spec.w, 512):
                c1 = min(c0 + 512, spec.w)
                nc.tensor.matmul(ps[:, c0:c1], lt[:], mask_sb[spec.name][:, c0:c1],
                                 start=True, stop=True)
            nc.scalar.copy(master[(spec.name, nm)][0:22, :], ps[:])

    # ---- stage 1: pair products -> master rows 32..41
    for spec in (HSPEC, GSPEC):
        name = spec.name
        w = spec.w
        is_g = spec is GSPEC
        packs = {}
        for nm in "ri":
            ps = sel_mm(sel_sb[f"SELP_{name}"], master[(name, nm)], 42, w)
            sb = pk.tile([10, w], FP32, tag=f"half{nm}")
            nc.scalar.copy(sb[:], ps[32:42, :])
            p0 = ps[0:10, :]
            if is_g:           # GPSIMD has no PSUM port
                p0sb = pk.tile([10, w], FP32, tag=f"p0{nm}")
                nc.scalar.copy(p0sb[:], ps[0:10, :])
                p0 = p0sb[:]
            packs[nm] = (p0, sb)
        cmul6(10, w,
              (packs["r"][0], packs["i"][0]),
              (packs["r"][1][:], packs["i"][1][:]),
              master[(name, "r")][M_PP:M_PP + 10, :],
              master[(name, "i")][M_PP:M_PP + 10, :],
              eng=nc.gpsimd if is_g else None)

    # ---- stage 2 (L1): tmp[64g+c] = tab_a[za]*tab_b[zb]
    #      stage 3 (L2): e_g = tmp * tab_c[zc]  -> [real(64); imag(64)] tiles
    EG = {}
    for spec in (HSPEC, GSPEC):
        name = spec.name
        w = spec.w
        # L1: one packed sel-mm -> 48 distinct pair-pair products
        is_g = spec is GSPEC
        tmp48 = {}
        for nm in "ri":
            ps = sel_mm(sel_sb[f"SELL1_{name}"], master[(name, nm)], 112, w)
            sb1 = pk.tile([48, w], FP32, tag=f"l1h{nm}")
            nc.scalar.copy(sb1[:], ps[64:112, :])
            p0 = ps[0:48, :]
            if is_g:
                p0sb = pk.tile([48, w], FP32, tag=f"l1p0{nm}")
                nc.scalar.copy(p0sb[:], ps[0:48, :])
                p0 = p0sb[:]
            t48 = pers.tile([48, w], FP32, tag=f"t48{name}{nm}",
                            name=f"t48{name}{nm}")
            tmp48[nm] = (p0, sb1, t48)
        cmul6(48, w,
              (tmp48["r"][0], tmp48["i"][0]),
              (tmp48["r"][1][:], tmp48["i"][1][:]),
              tmp48["r"][2][:], tmp48["i"][2][:],
              eng=nc.gpsimd if is_g else None)
        # L2 c-packs -> SBUF (ACT, transient PSUM)
        cp01 = {}
        cp2 = {}
        for nm in "ri":
            c01 = sel_mm(sel_sb[f"SELL2A_{name}"], master[(name, nm)], 128, w)
            c01sb = pk.tile([128, w], FP32, tag=f"c01{nm}")
            nc.scalar.copy(c01sb[:], c01[:])
            cp01[nm] = c01sb
            c2 = sel_mm(sel_sb[f"SELL2B_{name}"], master[(name, nm)], 64, w)
            c2sb = pk.tile([64, w], FP32, tag=f"c2{nm}")
            nc.scalar.copy(c2sb[:], c2[:])
            cp2[nm] = c2sb
        # REP matmuls: tmp48 -> per-group L2 src0 layout (PSUM; SBUF for G)
        rep = {}
        for nm in "ri":
            t48 = tmp48[nm][2]
            ra = sel_mm(sel_sb[f"REPA_{name}"], t48[:], 128, w)
            rb = sel_mm(sel_sb[f"REPB_{name}"], t48[:], 64, w)
            if spec is GSPEC:
                rasb = pk.tile([128, w], FP32, tag=f"ra{nm}")
                nc.scalar.copy(rasb[:], ra[:])
                ra = rasb
            # rb always to SBUF: the g2 set runs on GPSIMD (no PSUM port)
            rbsb = pk.tile([64, w], FP32, tag=f"rb{nm}")
            nc.scalar.copy(rbsb[:], rb[:])
            rb = rbsb
            rep[nm] = (ra, rb)
        e = [pers.tile([128, w], FP32, tag=f"e{g}{name}", name=f"e{g}{name}")
             for g in range(3)]
        for g in range(2):
            cmul6(64, w,
                  (rep["r"][0][64 * g:64 * g + 64, :],
                   rep["i"][0][64 * g:64 * g + 64, :]),
                  (cp01["r"][64 * g:64 * g + 64, :],
                   cp01["i"][64 * g:64 * g + 64, :]),
                  e[g][0:64, :], e[g][64:128, :],
                  eng=nc.gpsimd if spec is GSPEC else None)
        cmul6(64, w,
              (rep["r"][1][0:64, :], rep["i"][1][0:64, :]),
              (cp2["r"][:], cp2["i"][:]),
              e[2][0:64, :], e[2][64:128, :],
              eng=nc.gpsimd if spec is HSPEC else None)
        EG[name] = e

    # G-side stacked lhsT tiles: gA = [Gr; -Gi], gB = [Gi; Gr]
    gAB = []
    for g in range(3):
        eg = EG["G"][g]
        gA = pers.tile([128, P], FP32, tag=f"gA{g}")
        gB = pers.tile([128, P], FP32, tag=f"gB{g}")
        nc.scalar.copy(gA[0:64, :], eg[0:64, :])
        nc.scalar.mul(gA[64:128, :], eg[64:128, :], -1.0)
        nc.scalar.copy(gB[0:64, :], eg[64:128, :])
        nc.scalar.copy(gB[64:128, :], eg[0:64, :])
        gAB.append((gA, gB))

    # ---- stage 4: T matmuls, one K=128 matmul per (group, comp, chunk)
    Tps = {}
    Tsb = {}
    for g in range(3):
        gA, gB = gAB[g]
        hg = EG["H"][g]
        tr = psum_pool.tile([P, F], FP32, tag="ps")
        ti = psum_pool.tile([P, F], FP32, tag="ps")
        for c0 in range(0, F, 512):
            c1 = c0 + 512
            nc.tensor.matmul(tr[:, c0:c1], gA[:], hg[:, c0:c1],
                             start=True, stop=True)
            nc.tensor.matmul(ti[:, c0:c1], gB[:], hg[:, c0:c1],
                             start=True, stop=True)
        if g == 1:
            sr = pers.tile([P, F], FP32, tag="T1r")
            si = pers.tile([P, F], FP32, tag="T1i")
            nc.scalar.copy(sr[:], tr[:])
            nc.scalar.copy(si[:], ti[:])
            Tsb[g] = (sr, si)
        else:
            Tps[g] = (tr, ti)

    # p01 = T0*T1: f-split across DVE (cols 0:FS, T0 from PSUM) and GPSIMD
    # (cols FS:F, T0 tail evacuated to SBUF -- no PSUM port on GPSIMD)
    FS = 704
    p01r = pers.tile([P, F], FP32, tag="p01r")
    p01i = pers.tile([P, F], FP32, tag="p01i")
    t0tr = pers.tile([P, F - FS], FP32, tag="t0tr")
    t0ti = pers.tile([P, F - FS], FP32, tag="t0ti")
    nc.scalar.copy(t0tr[:], Tps[0][0][:, FS:F])
    nc.scalar.copy(t0ti[:], Tps[0][1][:, FS:F])
    cmul6(P, FS,
          (Tps[0][0][:, 0:FS], Tps[0][1][:, 0:FS]),
          (Tsb[1][0][:, 0:FS], Tsb[1][1][:, 0:FS]),
          p01r[:, 0:FS], p01i[:, 0:FS])
    cmul6(P, F - FS,
          (t0tr[:], t0ti[:]),
          (Tsb[1][0][:, FS:F], Tsb[1][1][:, FS:F]),
          p01r[:, FS:F], p01i[:, FS:F], eng=nc.gpsimd)
    t2r, t2i = Tps[2]

    scr2 = pers.tile([P, F], FP32, tag="ttr_scr")
    acc = pers.tile([P, 4], FP32, tag="acc")
    nc.vector.scalar_tensor_tensor(
        out=scr2[:], in0=p01r[:], scalar=1.0, in1=t2r[:],
        op0=OP.mult, op1=OP.mult, accum_out=acc[:, 0:1])
    nc.vector.scalar_tensor_tensor(
        out=scr2[:], in0=p01i[:], scalar=-1.0, in1=t2i[:],
        op0=OP.mult, op1=OP.mult, accum_out=acc[:, 1:2])
    nc.vector.scalar_tensor_tensor(
        out=scr2[:], in0=p01r[:], scalar=1.0, in1=t2i[:],
        op0=OP.mult, op1=OP.mult, accum_out=acc[:, 2:3])
    nc.vector.scalar_tensor_tensor(
        out=scr2[:], in0=p01i[:], scalar=1.0, in1=t2r[:],
        op0=OP.mult, op1=OP.mult, accum_out=acc[:, 3:4])

    ones = pers.tile([P, 1], FP32, tag="ones")
    nc.gpsimd.memset(ones[:], 1.0)
    accs = pers.tile([P, 2], FP32, tag="accs")
    nc.vector.tensor_add(accs[:, 0:1], acc[:, 0:1], acc[:, 1:2])
    nc.vector.tensor_add(accs[:, 1:2], acc[:, 2:3], acc[:, 3:4])
    red = psum_pool.tile([1, 2], FP32, tag="ps")
    nc.tensor.matmul(red[:], ones[:], accs[:], start=True, stop=True)
    perm = pers.tile([1, 2], FP32, tag="perm")
    nc.vector.tensor_copy(perm[:], red[:])

    # ---- classical term: prod(|A|^2) via sequential scan (fp32 prod semantics)
    sq1 = pers.tile([N, N], FP32, tag="sq1")
    sq2 = pers.tile([N, N], FP32, tag="sq2")
    nc.vector.tensor_mul(sq1[:], sbA["r"][:], sbA["r"][:])
    nc.vector.tensor_mul(sq2[:], sbA["i"][:], sbA["i"][:])
    nc.vector.tensor_add(sq1[:], sq1[:], sq2[:])
    dma(tens["sq_dram"][:, :], sq1[:])
    sqrow = pers.tile([1, N * N], FP32, tag="sqrow")
    dma(sqrow[:], tens["sq_dram"][:, :])
    scan = pers.tile([1, N * N], FP32, tag="scan")
    nc.vector.tensor_tensor_scan(
        out=scan[:], data0=sqrow[:], data1=sqrow[:], initial=1.0,
        op0=OP.mult, op1=OP.bypass)
    classical = pers.tile([1, 1], FP32, tag="classical")
    nc.vector.tensor_copy(classical[:], scan[:, N * N - 1:N * N])

    # ---- combine
    p2 = pers.tile([1, 2], FP32, tag="p2")
    nc.vector.tensor_mul(p2[:], perm[:], perm[:])
    pa2 = pers.tile([1, 1], FP32, tag="pa2")
    nc.vector.reduce_sum(pa2[:], p2[:], axis=mybir.AxisListType.X)
    res = pers.tile([1, 1], FP32, tag="res")
    nc.vector.tensor_scalar_mul(res[:], pa2[:], float(EMU * SCALE2))
    cl2 = pers.tile([1, 1], FP32, tag="cl2")
    nc.vector.tensor_scalar_mul(cl2[:], classical[:], float(1.0 - EMU))
    nc.vector.tensor_add(res[:], res[:], cl2[:])
    nc.vector.tensor_scalar_add(res[:], res[:], float(DARK))
    dma(tens["OUT"][:, :], res[:])

    ctx.close()


# ---------------------------------------------------------------- entry point
def kernel(A_real: np.ndarray, A_imag: np.ndarray) -> np.ndarray:
    B = A_real.shape[0]
    assert B == 8 and A_real.shape == (B, N, N)
    if "nc" not in _CACHE:
        _CACHE["nc"] = build_kernel()
    nc = _CACHE["nc"]
    consts = host_consts()
    in_maps = []
    for b in range(B):
        m = {"A_real": np.ascontiguousarray(A_real[b], dtype=np.float32),
             "A_imag": np.ascontiguousarray(A_imag[b], dtype=np.float32)}
        m.update(consts)
        in_maps.append(m)
    res = run_bass_kernel_spmd(nc, in_maps, list(range(B)))
    out = np.array([res.results[b]["OUT"].reshape(-1)[0] for b in range(B)],
                   dtype=np.float32)
    return out


if __name__ == "__main__":
    A_real = np.load("/tmp/A_real.npy")
    A_imag = np.load("/tmp/A_imag.npy")
    print(kernel(A_real, A_imag))

